# revision 18
# baseline (speedup 1.0000x reference)
"""Trainium2 Bass kernel for the 3-layer MLP encode/decode forward pass.

Computation (B = 65536):
    d_i = pinv(W_i)                       (host, negligible)
    h = lrelu(x @ W1.T)                   [B, 128]
    h = lrelu(h @ W2.T)                   [B, 64]
    h = h @ W3.T                          [B, 16]
    h = lrelu(h @ d3.T)                   [B, 64]   (folded: lrelu((d3@W3) @ h2))
    h = lrelu(h @ d2.T)                   [B, 128]
    out = h @ d1.T                        [B, 784]

Sharding: pure data-parallel — 8 cores x 8192 batch rows each; the tiny
weights (and host-side pinv) are replicated.

The kernel is HBM-bound: 2 x B x 784 elements of I/O vs ~0.4 GFLOP of
matmul per core.  All I/O and matmul operands are fp16 (fp32 PSUM
accumulation; end-to-end rel err ~6e-4, well inside the 2e-2 gate),
which halves the DMA traffic vs fp32 and doubles DVE copy throughput
for the 16-bit transpose tiles.

Per-core layout: activations are kept feature-major ([feat, batch]) so
TensorE contracts over features.  x is transposed on-chip via PE
transpose-mode.  The final layer swaps operand roles (stationary =
activation tile, moving = d1.T) so the output lands batch-major in
PSUM — no output transpose.

Pipelining: tiles are processed in pairs with the layer ladder emitted
layer-major across the pair (mm2 A, mm2 B, mm3 A, ...), so each
engine's in-order stream always has the sibling tile's work between a
matmul and the activation that consumes it.  DMA moves `dma_group`
512-row tiles per transfer (batch order inside a group is permuted;
the permutation cancels between input transposes and output writeback).
"""

import numpy as np

B = 65536
N_CORES = 8
B_LOC = B // N_CORES  # 8192
D0, D1, D2, D3 = 784, 128, 64, 16
KCH = 112          # 784 = 7 * 112 contraction chunks for layer 1
NKC = D0 // KCH    # 7
TILE = 512         # moving free dim per matmul (one fp32 PSUM bank)
SUB = 128          # batch sub-tile (partition dim of x / out tiles)
NSUB = TILE // SUB  # 4
HALF = D0 // 2     # 392

IO_DT = "float16"  # on-chip + DRAM dtype for x / weights / out


def _build_nc(b_loc=B_LOC, repeat=1, io_dt_name=IO_DT,
              in_dma_eng="sync", out_dma_eng="gpsimd", dma_group=4,
              xin_bufs=2, outp_bufs=2, xtp_bufs=2, acts_bufs=2,
              ocopy="split", xcopy="vector", staggered=False,
              dma_only=False, compute_only=False,
              no_xpose=False, xpose_only=False, dma_tx=False,
              act_dve=False):
    import contextlib
    import concourse.tile as tile
    from concourse import bacc, mybir

    dt16 = getattr(mybir.dt, io_dt_name)
    f32 = mybir.dt.float32
    LRELU = mybir.ActivationFunctionType.Lrelu
    COPY = mybir.ActivationFunctionType.Copy

    nc = bacc.Bacc(trn_type="TRN2", target_bir_lowering=False, debug=False,
                   num_devices=N_CORES)

    x = nc.declare_dram_parameter("x", [b_loc, D0], dt16, isOutput=False).ap()
    w1t = nc.declare_dram_parameter("w1t", [D0, D1], dt16, isOutput=False).ap()
    w2t = nc.declare_dram_parameter("w2t", [D1, D2], dt16, isOutput=False).ap()
    m3t = nc.declare_dram_parameter("m3t", [D2, D2], dt16, isOutput=False).ap()
    d2t = nc.declare_dram_parameter("d2t", [D2, D1], dt16, isOutput=False).ap()
    d1t = nc.declare_dram_parameter("d1t", [D1, D0], dt16, isOutput=False).ap()
    ident = nc.declare_dram_parameter("ident", [SUB, SUB], dt16, isOutput=False).ap()
    out = nc.declare_dram_parameter("out", [b_loc, D0], dt16, isOutput=True).ap()

    G = dma_group
    n_tiles = b_loc // TILE
    assert n_tiles % G == 0
    n_groups = n_tiles // G
    # row = grp*512*G + p*4*G + s  (4*G batch rows per partition per group)
    x_r = x.rearrange("(n p s) f -> n p (s f)", p=SUB, s=NSUB * G)
    out_r = out.rearrange("(n p s) f -> n p (s f)", p=SUB, s=NSUB * G)

    with tile.TileContext(nc, num_cores=N_CORES, pool_alloc_mode="stack") as tc:
        with (
            tc.tile_pool(name="consts", bufs=1) as consts,
            tc.tile_pool(name="xin", bufs=xin_bufs) as xin,
            tc.tile_pool(name="xtp", bufs=xtp_bufs) as xtp,
            tc.tile_pool(name="acts", bufs=acts_bufs) as acts,
            tc.tile_pool(name="outp", bufs=outp_bufs) as outp,
            tc.tile_pool(name="psT", bufs=2, space="PSUM") as psT,
            tc.tile_pool(name="psMM", bufs=2, space="PSUM") as psMM,
            tc.tile_pool(name="psO", bufs=2, space="PSUM") as psO,
        ):
            # --- constants ---
            w1t_sb = consts.tile([KCH, NKC, D1], dt16)
            nc.sync.dma_start(out=w1t_sb, in_=w1t.rearrange("(c p) m -> p c m", p=KCH))
            w2t_sb = consts.tile([D1, D2], dt16)
            nc.sync.dma_start(out=w2t_sb, in_=w2t)
            m3t_sb = consts.tile([D2, D2], dt16)
            nc.sync.dma_start(out=m3t_sb, in_=m3t)
            d2t_sb = consts.tile([D2, D1], dt16)
            nc.sync.dma_start(out=d2t_sb, in_=d2t)
            d1t_sb = consts.tile([D1, D0], dt16)
            nc.sync.dma_start(out=d1t_sb, in_=d1t)
            id_sb = consts.tile([SUB, SUB], dt16)
            nc.sync.dma_start(out=id_sb, in_=ident)

            if compute_only:
                x_const = consts.tile([SUB, NSUB * G, D0], dt16)
                nc.sync.dma_start(out=x_const, in_=x_r[0])

            xt_const = None
            if no_xpose:
                xt_const = []
                for c in range(NKC):
                    xc = consts.tile([KCH, TILE], dt16, name=f"xt_const{c}")
                    nc.sync.dma_start(out=xc, in_=x[c * KCH:(c + 1) * KCH, :TILE])
                    xt_const.append(xc)

            def xpose(x_sb, j):
                """Transpose subtile j to feature-major: 7 x [112, 512]."""
                if no_xpose:
                    return xt_const
                xt_sb = []
                for c in range(NKC):
                    tp = psT.tile([KCH, TILE], dt16, tag="psT")
                    for u in range(NSUB):
                        nc.tensor.transpose(
                            out=tp[:, u * SUB:(u + 1) * SUB],
                            in_=x_sb[:, j * NSUB + u, c * KCH:(c + 1) * KCH],
                            identity=id_sb,
                        )
                    xt = xtp.tile([KCH, TILE], dt16, tag=f"xt{j}_{c}", bufs=2)
                    if xcopy == "scalar":
                        nc.scalar.activation(out=xt, in_=tp, func=COPY)
                    else:
                        nc.vector.tensor_copy(xt, tp)
                    xt_sb.append(xt)
                return xt_sb

            def l1(xt_sb, j):
                """L1: 7 accumulating matmuls + lrelu -> h1 [128, 512]."""
                h1_ps = psMM.tile([D1, TILE], f32, tag="mm")
                for c in range(NKC):
                    nc.tensor.matmul(h1_ps, lhsT=w1t_sb[:, c, :], rhs=xt_sb[c],
                                     start=(c == 0), stop=(c == NKC - 1))
                h1_sb = acts.tile([D1, TILE], dt16, tag=f"h1_{j}", name="h1_sb",
                                  bufs=2)
                if act_dve:
                    nc.vector.tensor_copy(h1_sb, h1_ps)
                else:
                    nc.scalar.activation(out=h1_sb, in_=h1_ps, func=LRELU,
                                         alpha=0.01)
                return h1_sb

            def step(h_sb, w_sb, m, tag):
                """One ladder layer: [m, 512] = lrelu(w_sb.T @ h_sb)."""
                ps = psMM.tile([m, TILE], f32, tag="mm", name="ps")
                nc.tensor.matmul(ps, lhsT=w_sb, rhs=h_sb, start=True, stop=True)
                sb = acts.tile([m, TILE], dt16, tag=tag, name="sb", bufs=2)
                if act_dve:
                    nc.vector.tensor_copy(sb, ps)
                else:
                    nc.scalar.activation(out=sb, in_=ps, func=LRELU, alpha=0.01)
                return sb

            def l5(g2_sb, o_sb, j):
                """out = g2.T @ d1.T, batch-major via stationary swap."""
                for s in range(NSUB):
                    g2c = g2_sb[:, s * SUB:(s + 1) * SUB]
                    po = psO.tile([SUB, 1024], f32, tag="po")
                    nc.tensor.matmul(po[:, :HALF], lhsT=g2c, rhs=d1t_sb[:, :HALF],
                                     start=True, stop=True)
                    nc.tensor.matmul(po[:, 512:512 + HALF], lhsT=g2c,
                                     rhs=d1t_sb[:, HALF:], start=True, stop=True)
                    po_v = po.rearrange("p (b r) -> p b r", b=2)[:, :, :HALF]
                    o_v = o_sb[:, j * NSUB + s, :].rearrange("p (b r) -> p b r", b=2)
                    if ocopy == "scalar" or (ocopy == "split" and (j * NSUB + s) % 2 == 0):
                        nc.scalar.activation(out=o_v, in_=po_v, func=COPY)
                    else:
                        nc.vector.tensor_copy(o_v, po_v)

            rep_ctx = (tc.For_i(0, repeat, 1, staggered_reset=staggered)
                       if repeat > 1 else contextlib.nullcontext())
            with rep_ctx:
              for g in range(n_groups):
                # --- load G*512 rows in one DMA: [128, G*4, 784] fp16 ---
                if compute_only:
                    x_sb = x_const
                elif dma_only and dma_tx:
                    x_sb = None
                else:
                    x_sb = xin.tile([SUB, NSUB * G, D0], dt16, tag="x")
                    if in_dma_eng == "alt":
                        (nc.sync if g % 2 == 0 else nc.scalar).dma_start(
                            out=x_sb, in_=x_r[g])
                    else:
                        getattr(nc, in_dma_eng).dma_start(out=x_sb, in_=x_r[g])

                o_sb = outp.tile([SUB, NSUB * G, D0], dt16, tag="o")

                if dma_only:
                    if dma_tx:
                        for c in range(6):
                            xt_g = xtp.tile([SUB, TILE * G], dt16, tag=f"xtg{c}",
                                            bufs=2)
                            nc.sync.dma_start(
                                out=xt_g,
                                in_=x[g * TILE * G:(g + 1) * TILE * G,
                                      c * SUB:(c + 1) * SUB],
                                transpose=True)
                            nc.vector.tensor_copy(
                                o_sb.rearrange("p a b -> p (a b)")
                                    [:, c * TILE * G:(c + 1) * TILE * G], xt_g)
                    else:
                        nc.vector.tensor_copy(o_sb, x_sb)
                else:
                    # --- phase-major over the group: long dense matmul
                    # bursts keep the PE HAM clock-gate open ---
                    xts = [xpose(x_sb, j) for j in range(G)]
                    if xpose_only:
                        nc.vector.tensor_copy(o_sb, x_sb)
                    else:
                        h1s = [l1(xts[j], j) for j in range(G)]
                        h2s = [step(h1s[j], w2t_sb, D2, f"h2_{j}")
                               for j in range(G)]
                        g3s = [step(h2s[j], m3t_sb, D2, f"g3_{j}")
                               for j in range(G)]
                        g2s = [step(g3s[j], d2t_sb, D1, f"g2_{j}")
                               for j in range(G)]
                        for j in range(G):
                            l5(g2s[j], o_sb, j)

                if not compute_only:
                    getattr(nc, out_dma_eng).dma_start(out=out_r[g], in_=o_sb)

              if compute_only:
                nc.sync.dma_start(out=out_r[0], in_=o_sb)

    nc.finalize()
    return nc


def _host_weights(W1, W2, W3):
    def pinv(W):
        u, s, vh = np.linalg.svd(W.astype(np.float64), full_matrices=False)
        return (vh.T * (1.0 / s)) @ u.T

    d1, d2, d3 = pinv(W1), pinv(W2), pinv(W3)
    f = np.float16
    return {
        "w1t": np.ascontiguousarray(W1.T, dtype=f),
        "w2t": np.ascontiguousarray(W2.T, dtype=f),
        "m3t": np.ascontiguousarray((d3 @ W3.astype(np.float64)).T, dtype=f),
        "d2t": np.ascontiguousarray(d2.T, dtype=f),
        "d1t": np.ascontiguousarray(d1.T, dtype=f),
        "ident": np.eye(SUB, dtype=f),
    }


def _in_maps(x, W1, W2, W3):
    x = np.ascontiguousarray(x, dtype=np.float16)
    w = _host_weights(np.asarray(W1), np.asarray(W2), np.asarray(W3))
    return [{"x": x[i * B_LOC:(i + 1) * B_LOC], **w} for i in range(N_CORES)]


_NC_CACHE = {}


def _get_nc(key=()):
    if key not in _NC_CACHE:
        _NC_CACHE[key] = _build_nc(B_LOC)
    return _NC_CACHE[key]


def kernel(x, W1, W2, W3):
    from concourse.bass_utils import run_bass_kernel_spmd

    in_maps = _in_maps(x, W1, W2, W3)
    nc = _get_nc()
    res = run_bass_kernel_spmd(nc, in_maps, core_ids=list(range(N_CORES)))
    return np.concatenate(
        [res.results[i]["out"] for i in range(N_CORES)], axis=0
    ).astype(np.float32)


# revision 20
# speedup vs baseline: 1.0098x; 1.0098x over previous
"""Trainium2 Bass kernel for the 3-layer MLP encode/decode forward pass.

Computation (B = 65536):
    d_i = pinv(W_i)                       (host, negligible)
    h = lrelu(x @ W1.T)                   [B, 128]
    h = lrelu(h @ W2.T)                   [B, 64]
    h = h @ W3.T                          [B, 16]
    h = lrelu(h @ d3.T)                   [B, 64]   (folded: lrelu((d3@W3) @ h2))
    h = lrelu(h @ d2.T)                   [B, 128]
    out = h @ d1.T                        [B, 784]

Sharding: pure data-parallel — 8 cores x 8192 batch rows each; the tiny
weights (and host-side pinv) are replicated.

All I/O and matmul operands are fp16 (fp32 PSUM accumulation;
end-to-end rel err ~6e-4, well inside the 2e-2 gate), which halves the
DMA traffic vs fp32.  The kernel is jointly bound by HBM traffic
(2 x 12.9 MB/core/pass) and the PE sequencer's LDWEIGHTS+MATMUL issue
stream (~46 pairs per 512-row tile).

Per-core layout: activations are kept feature-major ([feat, batch]) so
TensorE contracts over features.  x is transposed on-chip via PE
transpose-mode.  The final layer swaps operand roles (stationary =
activation tile, moving = d1.T) so the output lands batch-major in
PSUM — no output transpose.

DMA: one 0.8MB transfer per 512-row tile each way ([128, 4*784] fp16
with 4 batch rows per partition — batch order inside a tile is permuted,
which cancels between the input transposes and the output writeback).
"""

import numpy as np

B = 65536
N_CORES = 8
B_LOC = B // N_CORES  # 8192
D0, D1, D2, D3 = 784, 128, 64, 16
KCH = 112          # 784 = 7 * 112 contraction chunks for layer 1
NKC = D0 // KCH    # 7
TILE = 512         # moving free dim per matmul (one fp32 PSUM bank)
SUB = 128          # batch sub-tile (partition dim of x / out tiles)
NSUB = TILE // SUB  # 4
HALF = D0 // 2     # 392

IO_DT = "float16"  # on-chip + DRAM dtype for x / weights / out


def _build_nc(b_loc=B_LOC, repeat=1, io_dt_name=IO_DT,
              in_dma_eng="sync", out_dma_eng="sync",
              xin_bufs=4, outp_bufs=4, xtp_bufs=14, acts_bufs=2,
              ocopy="split", xcopy="vector", staggered=False,
              dma_only=False, compute_only=False):
    import contextlib
    import concourse.tile as tile
    from concourse import bacc, mybir

    dt16 = getattr(mybir.dt, io_dt_name)
    f32 = mybir.dt.float32
    LRELU = mybir.ActivationFunctionType.Lrelu
    COPY = mybir.ActivationFunctionType.Copy

    nc = bacc.Bacc(trn_type="TRN2", target_bir_lowering=False, debug=False,
                   num_devices=N_CORES)

    x = nc.declare_dram_parameter("x", [b_loc, D0], dt16, isOutput=False).ap()
    w1t = nc.declare_dram_parameter("w1t", [D0, D1], dt16, isOutput=False).ap()
    w2t = nc.declare_dram_parameter("w2t", [D1, D2], dt16, isOutput=False).ap()
    m3t = nc.declare_dram_parameter("m3t", [D2, D2], dt16, isOutput=False).ap()
    d2t = nc.declare_dram_parameter("d2t", [D2, D1], dt16, isOutput=False).ap()
    d1t = nc.declare_dram_parameter("d1t", [D1, D0], dt16, isOutput=False).ap()
    ident = nc.declare_dram_parameter("ident", [SUB, SUB], dt16, isOutput=False).ap()
    out = nc.declare_dram_parameter("out", [b_loc, D0], dt16, isOutput=True).ap()

    n_tiles = b_loc // TILE
    # row = tile*512 + p*4 + s  (4 rows per partition -> one 0.8MB DMA per tile)
    x_r = x.rearrange("(n p s) f -> n p (s f)", p=SUB, s=NSUB)
    out_r = out.rearrange("(n p s) f -> n p (s f)", p=SUB, s=NSUB)

    with tile.TileContext(nc, num_cores=N_CORES, pool_alloc_mode="stack") as tc:
        with (
            tc.tile_pool(name="consts", bufs=1) as consts,
            tc.tile_pool(name="xin", bufs=xin_bufs) as xin,
            tc.tile_pool(name="xtp", bufs=xtp_bufs) as xtp,
            tc.tile_pool(name="acts", bufs=acts_bufs) as acts,
            tc.tile_pool(name="outp", bufs=outp_bufs) as outp,
            tc.tile_pool(name="psT", bufs=2, space="PSUM") as psT,
            tc.tile_pool(name="psMM", bufs=2, space="PSUM") as psMM,
            tc.tile_pool(name="psO", bufs=2, space="PSUM") as psO,
        ):
            # --- constants ---
            w1t_sb = consts.tile([KCH, NKC, D1], dt16)
            nc.sync.dma_start(out=w1t_sb, in_=w1t.rearrange("(c p) m -> p c m", p=KCH))
            w2t_sb = consts.tile([D1, D2], dt16)
            nc.sync.dma_start(out=w2t_sb, in_=w2t)
            m3t_sb = consts.tile([D2, D2], dt16)
            nc.sync.dma_start(out=m3t_sb, in_=m3t)
            d2t_sb = consts.tile([D2, D1], dt16)
            nc.sync.dma_start(out=d2t_sb, in_=d2t)
            d1t_sb = consts.tile([D1, D0], dt16)
            nc.sync.dma_start(out=d1t_sb, in_=d1t)
            id_sb = consts.tile([SUB, SUB], dt16)
            nc.sync.dma_start(out=id_sb, in_=ident)

            if compute_only:
                x_const = consts.tile([SUB, NSUB, D0], dt16)
                nc.sync.dma_start(out=x_const, in_=x_r[0])

            rep_ctx = (tc.For_i(0, repeat, 1, staggered_reset=staggered)
                       if repeat > 1 else contextlib.nullcontext())
            with rep_ctx:
              for t in range(n_tiles):
                # --- load 512 rows in one DMA: [128, 4, 784] fp16 ---
                if compute_only:
                    x_sb = x_const
                else:
                    x_sb = xin.tile([SUB, NSUB, D0], dt16, tag="x")
                    if in_dma_eng == "alt":
                        (nc.sync if t % 2 == 0 else nc.scalar).dma_start(
                            out=x_sb, in_=x_r[t])
                    else:
                        getattr(nc, in_dma_eng).dma_start(out=x_sb, in_=x_r[t])

                if dma_only:
                    o_sb = outp.tile([SUB, NSUB, D0], dt16, tag="o")
                    nc.vector.tensor_copy(o_sb, x_sb)
                    getattr(nc, out_dma_eng).dma_start(out=out_r[t], in_=o_sb)
                    continue

                # --- PE-transpose to feature-major: 7 chunks of [112, 512] ---
                xt_sb = []
                for c in range(NKC):
                    tp = psT.tile([KCH, TILE], dt16, tag="psT")
                    for s in range(NSUB):
                        nc.tensor.transpose(
                            out=tp[:, s * SUB:(s + 1) * SUB],
                            in_=x_sb[:, s, c * KCH:(c + 1) * KCH],
                            identity=id_sb,
                        )
                    xt = xtp.tile([KCH, TILE], dt16, tag="xt")
                    if xcopy == "scalar":
                        nc.scalar.activation(out=xt, in_=tp, func=COPY)
                    else:
                        nc.vector.tensor_copy(xt, tp)
                    xt_sb.append(xt)

                # --- L1: h1 = lrelu(W1 @ xT)  [128, 512] ---
                h1_ps = psMM.tile([D1, TILE], f32, tag="mm")
                for c in range(NKC):
                    nc.tensor.matmul(h1_ps, lhsT=w1t_sb[:, c, :], rhs=xt_sb[c],
                                     start=(c == 0), stop=(c == NKC - 1))
                h1_sb = acts.tile([D1, TILE], dt16, tag="h1")
                nc.scalar.activation(out=h1_sb, in_=h1_ps, func=LRELU, alpha=0.01)

                # --- L2: h2 = lrelu(W2 @ h1)  [64, 512] ---
                h2_ps = psMM.tile([D2, TILE], f32, tag="mm")
                nc.tensor.matmul(h2_ps, lhsT=w2t_sb, rhs=h1_sb,
                                 start=True, stop=True)
                h2_sb = acts.tile([D2, TILE], dt16, tag="h2")
                nc.scalar.activation(out=h2_sb, in_=h2_ps, func=LRELU, alpha=0.01)

                # --- L3 folded: g3 = lrelu((d3 @ W3) @ h2)  [64, 512] ---
                g3_ps = psMM.tile([D2, TILE], f32, tag="mm")
                nc.tensor.matmul(g3_ps, lhsT=m3t_sb, rhs=h2_sb,
                                 start=True, stop=True)
                g3_sb = acts.tile([D2, TILE], dt16, tag="g3")
                nc.scalar.activation(out=g3_sb, in_=g3_ps, func=LRELU, alpha=0.01)

                # --- L4: g2 = lrelu(d2 @ g3)  [128, 512] ---
                g2_ps = psMM.tile([D1, TILE], f32, tag="mm")
                nc.tensor.matmul(g2_ps, lhsT=d2t_sb, rhs=g3_sb,
                                 start=True, stop=True)
                g2_sb = acts.tile([D1, TILE], dt16, tag="g2")
                nc.scalar.activation(out=g2_sb, in_=g2_ps, func=LRELU, alpha=0.01)

                # --- L5: out = g2.T @ d1.T, batch-major via stationary swap.
                # Two matmuls into one 2-bank PSUM tile ([:, :392] in bank 0,
                # [:, 512:904] in bank 1), one strided copy out. ---
                o_sb = outp.tile([SUB, NSUB, D0], dt16, tag="o")
                for s in range(NSUB):
                    g2c = g2_sb[:, s * SUB:(s + 1) * SUB]
                    po = psO.tile([SUB, 1024], f32, tag="po")
                    nc.tensor.matmul(po[:, :HALF], lhsT=g2c, rhs=d1t_sb[:, :HALF],
                                     start=True, stop=True)
                    nc.tensor.matmul(po[:, 512:512 + HALF], lhsT=g2c,
                                     rhs=d1t_sb[:, HALF:], start=True, stop=True)
                    po_v = po.rearrange("p (b r) -> p b r", b=2)[:, :, :HALF]
                    o_v = o_sb[:, s, :].rearrange("p (b r) -> p b r", b=2)
                    if ocopy == "scalar" or (ocopy == "split" and s % 2 == 0):
                        nc.scalar.activation(out=o_v, in_=po_v, func=COPY)
                    else:
                        nc.vector.tensor_copy(o_v, po_v)
                getattr(nc, out_dma_eng).dma_start(out=out_r[t], in_=o_sb)

              if compute_only:
                nc.sync.dma_start(out=out_r[0], in_=o_sb)

    nc.finalize()
    return nc


def _host_weights(W1, W2, W3):
    def pinv(W):
        u, s, vh = np.linalg.svd(W.astype(np.float64), full_matrices=False)
        return (vh.T * (1.0 / s)) @ u.T

    d1, d2, d3 = pinv(W1), pinv(W2), pinv(W3)
    f = np.float16
    return {
        "w1t": np.ascontiguousarray(W1.T, dtype=f),
        "w2t": np.ascontiguousarray(W2.T, dtype=f),
        "m3t": np.ascontiguousarray((d3 @ W3.astype(np.float64)).T, dtype=f),
        "d2t": np.ascontiguousarray(d2.T, dtype=f),
        "d1t": np.ascontiguousarray(d1.T, dtype=f),
        "ident": np.eye(SUB, dtype=f),
    }


def _in_maps(x, W1, W2, W3):
    x = np.ascontiguousarray(x, dtype=np.float16)
    w = _host_weights(np.asarray(W1), np.asarray(W2), np.asarray(W3))
    return [{"x": x[i * B_LOC:(i + 1) * B_LOC], **w} for i in range(N_CORES)]


_NC_CACHE = {}


def _get_nc(key=()):
    if key not in _NC_CACHE:
        _NC_CACHE[key] = _build_nc(B_LOC)
    return _NC_CACHE[key]


def kernel(x, W1, W2, W3):
    from concourse.bass_utils import run_bass_kernel_spmd

    in_maps = _in_maps(x, W1, W2, W3)
    nc = _get_nc()
    res = run_bass_kernel_spmd(nc, in_maps, core_ids=list(range(N_CORES)))
    return np.concatenate(
        [res.results[i]["out"] for i in range(N_CORES)], axis=0
    ).astype(np.float32)


# revision 21
# speedup vs baseline: 1.0677x; 1.0573x over previous
"""Trainium2 Bass kernel for the 3-layer MLP encode/decode forward pass.

Computation (B = 65536):
    d_i = pinv(W_i)                       (host, negligible)
    h = lrelu(x @ W1.T)                   [B, 128]
    h = lrelu(h @ W2.T)                   [B, 64]
    h = h @ W3.T                          [B, 16]
    h = lrelu(h @ d3.T)                   [B, 64]   (folded: lrelu((d3@W3) @ h2))
    h = lrelu(h @ d2.T)                   [B, 128]
    out = h @ d1.T                        [B, 784]

Sharding: pure data-parallel — 8 cores x 8192 batch rows each; the tiny
weights (and host-side pinv) are replicated.

All I/O and matmul operands are fp16 (fp32 PSUM accumulation;
end-to-end rel err ~6e-4, well inside the 2e-2 gate), which halves the
DMA traffic vs fp32.  The kernel is jointly bound by HBM traffic
(2 x 12.9 MB/core/pass) and the PE sequencer's LDWEIGHTS+MATMUL issue
stream, so the layer-1 contraction is chunked 6 x 128 + 16: the six
128-column chunks are PE-transposed on chip (24 transpose issues per
512-row tile instead of 28, and 128-wide fp16 stationaries take the
fast-weight-load path), while the 16-column tail is shipped by the
host launcher already feature-major (xtail), column-permuted to match
the batch permutation the on-chip transposes produce.

Per-core layout: activations are kept feature-major ([feat, batch]) so
TensorE contracts over features.  Transposes land two 128-feature
chunks per fp16 PSUM bank so one DVE copy drains 1024 columns.  The
final layer swaps operand roles (stationary = activation tile, moving
= d1.T) so the output lands batch-major in PSUM — no output transpose.

DMA: per 512-row tile, one 0.75MB main load + 16KB tail load in, one
0.8MB store out (4 batch rows per partition — batch order inside a
tile is permuted; the permutation cancels between the input transposes
/ permuted tail and the output writeback).
"""

import numpy as np

B = 65536
N_CORES = 8
B_LOC = B // N_CORES  # 8192
D0, D1, D2, D3 = 784, 128, 64, 16
DM = 768           # 6 x 128 main feature chunks
NKC = DM // 128    # 6
TAIL = D0 - DM     # 16
TILE = 512         # moving free dim per matmul (one fp32 PSUM bank)
SUB = 128          # batch sub-tile (partition dim of x / out tiles)
NSUB = TILE // SUB  # 4
HALF = D0 // 2     # 392

IO_DT = "float16"  # on-chip + DRAM dtype for x / weights / out


def _build_nc(b_loc=B_LOC, repeat=1, io_dt_name=IO_DT,
              in_dma_eng="sync", out_dma_eng="sync",
              xin_bufs=4, outp_bufs=4, xtp_bufs=8, acts_bufs=2,
              ocopy="split", xcopy="vector", staggered=False,
              dma_only=False, compute_only=False):
    import contextlib
    import concourse.tile as tile
    from concourse import bacc, mybir

    dt16 = getattr(mybir.dt, io_dt_name)
    f32 = mybir.dt.float32
    LRELU = mybir.ActivationFunctionType.Lrelu
    COPY = mybir.ActivationFunctionType.Copy

    nc = bacc.Bacc(trn_type="TRN2", target_bir_lowering=False, debug=False,
                   num_devices=N_CORES)

    xm = nc.declare_dram_parameter("xm", [b_loc, DM], dt16, isOutput=False).ap()
    xtail = nc.declare_dram_parameter("xtail", [TAIL, b_loc], dt16,
                                      isOutput=False).ap()
    w1t = nc.declare_dram_parameter("w1t", [DM, D1], dt16, isOutput=False).ap()
    w1l = nc.declare_dram_parameter("w1l", [TAIL, D1], dt16, isOutput=False).ap()
    w2t = nc.declare_dram_parameter("w2t", [D1, D2], dt16, isOutput=False).ap()
    m3t = nc.declare_dram_parameter("m3t", [D2, D2], dt16, isOutput=False).ap()
    d2t = nc.declare_dram_parameter("d2t", [D2, D1], dt16, isOutput=False).ap()
    d1t = nc.declare_dram_parameter("d1t", [D1, D0], dt16, isOutput=False).ap()
    ident = nc.declare_dram_parameter("ident", [SUB, SUB], dt16, isOutput=False).ap()
    out = nc.declare_dram_parameter("out", [b_loc, D0], dt16, isOutput=True).ap()

    n_tiles = b_loc // TILE
    # row = tile*512 + p*4 + s  (4 rows per partition -> one 0.75MB DMA/tile);
    # xtail is host-permuted so its natural column order matches col = s*128+p.
    x_r = xm.rearrange("(n p s) f -> n p (s f)", p=SUB, s=NSUB)
    out_r = out.rearrange("(n p s) f -> n p (s f)", p=SUB, s=NSUB)
    xt_r = xtail.rearrange("q (n c) -> n q c", c=TILE)

    with tile.TileContext(nc, num_cores=N_CORES, pool_alloc_mode="stack") as tc:
        with (
            tc.tile_pool(name="consts", bufs=1) as consts,
            tc.tile_pool(name="xin", bufs=xin_bufs) as xin,
            tc.tile_pool(name="xtp", bufs=xtp_bufs) as xtp,
            tc.tile_pool(name="acts", bufs=acts_bufs) as acts,
            tc.tile_pool(name="outp", bufs=outp_bufs) as outp,
            tc.tile_pool(name="psT", bufs=2, space="PSUM") as psT,
            tc.tile_pool(name="psMM", bufs=2, space="PSUM") as psMM,
            tc.tile_pool(name="psO", bufs=2, space="PSUM") as psO,
        ):
            # --- constants ---
            w1t_sb = consts.tile([SUB, NKC, D1], dt16)
            nc.sync.dma_start(out=w1t_sb, in_=w1t.rearrange("(c p) m -> p c m", p=SUB))
            w1l_sb = consts.tile([TAIL, D1], dt16)
            nc.sync.dma_start(out=w1l_sb, in_=w1l)
            w2t_sb = consts.tile([D1, D2], dt16)
            nc.sync.dma_start(out=w2t_sb, in_=w2t)
            m3t_sb = consts.tile([D2, D2], dt16)
            nc.sync.dma_start(out=m3t_sb, in_=m3t)
            d2t_sb = consts.tile([D2, D1], dt16)
            nc.sync.dma_start(out=d2t_sb, in_=d2t)
            d1t_sb = consts.tile([D1, D0], dt16)
            nc.sync.dma_start(out=d1t_sb, in_=d1t)
            id_sb = consts.tile([SUB, SUB], dt16)
            nc.sync.dma_start(out=id_sb, in_=ident)

            if compute_only:
                x_const = consts.tile([SUB, NSUB, DM], dt16)
                nc.sync.dma_start(out=x_const, in_=x_r[0])
                xl_const = consts.tile([TAIL, TILE], dt16)
                nc.sync.dma_start(out=xl_const, in_=xt_r[0])

            rep_ctx = (tc.For_i(0, repeat, 1, staggered_reset=staggered)
                       if repeat > 1 else contextlib.nullcontext())
            with rep_ctx:
              for t in range(n_tiles):
                # --- load 512 rows: [128, 4, 768] main + [16, 512] tail ---
                if compute_only:
                    x_sb, xl_sb = x_const, xl_const
                else:
                    x_sb = xin.tile([SUB, NSUB, DM], dt16, tag="x")
                    xl_sb = xin.tile([TAIL, TILE], dt16, tag="xl")
                    if in_dma_eng == "alt":
                        eng = nc.sync if t % 2 == 0 else nc.scalar
                    else:
                        eng = getattr(nc, in_dma_eng)
                    eng.dma_start(out=x_sb, in_=x_r[t])
                    eng.dma_start(out=xl_sb, in_=xt_r[t])

                if dma_only:
                    o_sb = outp.tile([SUB, NSUB, D0], dt16, tag="o")
                    nc.vector.tensor_copy(
                        o_sb.rearrange("p a b -> p (a b)")[:, :NSUB * DM],
                        x_sb.rearrange("p a b -> p (a b)"))
                    getattr(nc, out_dma_eng).dma_start(out=out_r[t], in_=o_sb)
                    continue

                # --- PE-transpose to feature-major: 6 chunks of [128, 512],
                # two chunks per fp16 PSUM bank -> 3 pair copies ---
                xt_sb = []
                for cp in range(NKC // 2):
                    tp = psT.tile([SUB, 2, TILE], dt16, tag="psT")
                    for h in range(2):
                        c = 2 * cp + h
                        for s in range(NSUB):
                            nc.tensor.transpose(
                                out=tp[:, h, s * SUB:(s + 1) * SUB],
                                in_=x_sb[:, s, c * SUB:(c + 1) * SUB],
                                identity=id_sb,
                            )
                    xt = xtp.tile([SUB, 2, TILE], dt16, tag="xt")
                    if xcopy == "scalar":
                        nc.scalar.activation(out=xt, in_=tp, func=COPY)
                    else:
                        nc.vector.tensor_copy(xt, tp)
                    xt_sb.append(xt)

                # --- L1: h1 = lrelu(W1 @ xT)  [128, 512] (6 main + tail) ---
                h1_ps = psMM.tile([D1, TILE], f32, tag="mm")
                for c in range(NKC):
                    nc.tensor.matmul(h1_ps, lhsT=w1t_sb[:, c, :],
                                     rhs=xt_sb[c // 2][:, c % 2, :],
                                     start=(c == 0), stop=False)
                nc.tensor.matmul(h1_ps, lhsT=w1l_sb, rhs=xl_sb,
                                 start=False, stop=True)
                h1_sb = acts.tile([D1, TILE], dt16, tag="h1")
                nc.scalar.activation(out=h1_sb, in_=h1_ps, func=LRELU, alpha=0.01)

                # --- L2: h2 = lrelu(W2 @ h1)  [64, 512] ---
                h2_ps = psMM.tile([D2, TILE], f32, tag="mm")
                nc.tensor.matmul(h2_ps, lhsT=w2t_sb, rhs=h1_sb,
                                 start=True, stop=True)
                h2_sb = acts.tile([D2, TILE], dt16, tag="h2")
                nc.scalar.activation(out=h2_sb, in_=h2_ps, func=LRELU, alpha=0.01)

                # --- L3 folded: g3 = lrelu((d3 @ W3) @ h2)  [64, 512] ---
                g3_ps = psMM.tile([D2, TILE], f32, tag="mm")
                nc.tensor.matmul(g3_ps, lhsT=m3t_sb, rhs=h2_sb,
                                 start=True, stop=True)
                g3_sb = acts.tile([D2, TILE], dt16, tag="g3")
                nc.scalar.activation(out=g3_sb, in_=g3_ps, func=LRELU, alpha=0.01)

                # --- L4: g2 = lrelu(d2 @ g3)  [128, 512] ---
                g2_ps = psMM.tile([D1, TILE], f32, tag="mm")
                nc.tensor.matmul(g2_ps, lhsT=d2t_sb, rhs=g3_sb,
                                 start=True, stop=True)
                g2_sb = acts.tile([D1, TILE], dt16, tag="g2")
                nc.scalar.activation(out=g2_sb, in_=g2_ps, func=LRELU, alpha=0.01)

                # --- L5: out = g2.T @ d1.T, batch-major via stationary swap.
                # Two matmuls into one 2-bank PSUM tile ([:, :392] in bank 0,
                # [:, 512:904] in bank 1), one strided copy out. ---
                o_sb = outp.tile([SUB, NSUB, D0], dt16, tag="o")
                for s in range(NSUB):
                    g2c = g2_sb[:, s * SUB:(s + 1) * SUB]
                    po = psO.tile([SUB, 1024], f32, tag="po")
                    nc.tensor.matmul(po[:, :HALF], lhsT=g2c, rhs=d1t_sb[:, :HALF],
                                     start=True, stop=True)
                    nc.tensor.matmul(po[:, 512:512 + HALF], lhsT=g2c,
                                     rhs=d1t_sb[:, HALF:], start=True, stop=True)
                    po_v = po.rearrange("p (b r) -> p b r", b=2)[:, :, :HALF]
                    o_v = o_sb[:, s, :].rearrange("p (b r) -> p b r", b=2)
                    if ocopy == "scalar" or (ocopy == "split" and s % 2 == 0):
                        nc.scalar.activation(out=o_v, in_=po_v, func=COPY)
                    else:
                        nc.vector.tensor_copy(o_v, po_v)
                getattr(nc, out_dma_eng).dma_start(out=out_r[t], in_=o_sb)

              if compute_only:
                nc.sync.dma_start(out=out_r[0], in_=o_sb)

    nc.finalize()
    return nc


def _host_weights(W1, W2, W3):
    def pinv(W):
        u, s, vh = np.linalg.svd(W.astype(np.float64), full_matrices=False)
        return (vh.T * (1.0 / s)) @ u.T

    d1, d2, d3 = pinv(W1), pinv(W2), pinv(W3)
    f = np.float16
    w1tf = W1.T  # [784, 128]
    return {
        "w1t": np.ascontiguousarray(w1tf[:DM], dtype=f),
        "w1l": np.ascontiguousarray(w1tf[DM:], dtype=f),
        "w2t": np.ascontiguousarray(W2.T, dtype=f),
        "m3t": np.ascontiguousarray((d3 @ W3.astype(np.float64)).T, dtype=f),
        "d2t": np.ascontiguousarray(d2.T, dtype=f),
        "d1t": np.ascontiguousarray(d1.T, dtype=f),
        "ident": np.eye(SUB, dtype=f),
    }


def _in_maps(x, W1, W2, W3):
    x = np.asarray(x, dtype=np.float16)
    w = _host_weights(np.asarray(W1), np.asarray(W2), np.asarray(W3))
    n_tiles = B_LOC // TILE
    maps = []
    for i in range(N_CORES):
        xs = x[i * B_LOC:(i + 1) * B_LOC]
        # tail, feature-major, columns permuted to the transpose order:
        # on-chip column s*128+p of tile t holds batch row t*512 + p*4 + s.
        tl = np.ascontiguousarray(xs[:, DM:].T)          # [16, 8192] natural
        tl = tl.reshape(TAIL, n_tiles, SUB, NSUB)        # [q, t, p, s]
        tl = np.ascontiguousarray(tl.transpose(0, 1, 3, 2))  # [q, t, s, p]
        maps.append({
            "xm": np.ascontiguousarray(xs[:, :DM]),
            "xtail": tl.reshape(TAIL, B_LOC),
            **w,
        })
    return maps


_NC_CACHE = {}


def _get_nc(key=()):
    if key not in _NC_CACHE:
        _NC_CACHE[key] = _build_nc(B_LOC)
    return _NC_CACHE[key]


def kernel(x, W1, W2, W3):
    from concourse.bass_utils import run_bass_kernel_spmd

    in_maps = _in_maps(x, W1, W2, W3)
    nc = _get_nc()
    res = run_bass_kernel_spmd(nc, in_maps, core_ids=list(range(N_CORES)))
    return np.concatenate(
        [res.results[i]["out"] for i in range(N_CORES)], axis=0
    ).astype(np.float32)


# revision 22
# speedup vs baseline: 1.0994x; 1.0296x over previous
"""Trainium2 Bass kernel for the 3-layer MLP encode/decode forward pass.

Computation (B = 65536):
    d_i = pinv(W_i)                       (host, negligible)
    h = lrelu(x @ W1.T)                   [B, 128]
    h = lrelu(h @ W2.T)                   [B, 64]
    h = h @ W3.T                          [B, 16]
    h = lrelu(h @ d3.T)                   [B, 64]   (folded: lrelu((d3@W3) @ h2))
    h = lrelu(h @ d2.T)                   [B, 128]
    out = h @ d1.T                        [B, 784]

Sharding: pure data-parallel — 8 cores x 8192 batch rows each; the tiny
weights (and host-side pinv) are replicated.

All I/O and matmul operands are fp16 (fp32 PSUM accumulation;
end-to-end rel err ~6e-4, well inside the 2e-2 gate), which halves the
DMA traffic vs fp32.  The kernel is jointly bound by HBM traffic
(2 x 12.9 MB/core/pass) and the PE sequencer's LDWEIGHTS+MATMUL issue
stream, so the layer-1 contraction is chunked 6 x 128 + 16: the six
128-column chunks are PE-transposed on chip (24 transpose issues per
512-row tile instead of 28, and 128-wide fp16 stationaries take the
fast-weight-load path), while the 16-column tail is shipped by the
host launcher already feature-major (xtail), column-permuted to match
the batch permutation the on-chip transposes produce.

Per-core layout: activations are kept feature-major ([feat, batch]) so
TensorE contracts over features.  Transposes land two 128-feature
chunks per fp16 PSUM bank so one DVE copy drains 1024 columns.  The
final layer swaps operand roles (stationary = activation tile, moving
= d1.T) so the output lands batch-major in PSUM — no output transpose.

DMA: per 512-row tile, one 0.75MB main load + 16KB tail load in, one
0.8MB store out (4 batch rows per partition — batch order inside a
tile is permuted; the permutation cancels between the input transposes
/ permuted tail and the output writeback).
"""

import numpy as np

B = 65536
N_CORES = 8
B_LOC = B // N_CORES  # 8192
D0, D1, D2, D3 = 784, 128, 64, 16
DM = 768           # 6 x 128 main feature chunks
NKC = DM // 128    # 6
TAIL = D0 - DM     # 16
TILE = 512         # moving free dim per matmul (one fp32 PSUM bank)
SUB = 128          # batch sub-tile (partition dim of x / out tiles)
NSUB = TILE // SUB  # 4
HALF = D0 // 2     # 392

IO_DT = "float16"  # on-chip + DRAM dtype for x / weights / out


def _build_nc(b_loc=B_LOC, repeat=1, io_dt_name=IO_DT,
              in_dma_eng="sync", out_dma_eng="sync",
              xin_bufs=4, outp_bufs=4, xtp_bufs=8, acts_bufs=2,
              ocopy="split", xcopy="vector", staggered=False,
              dma_only=False, compute_only=False):
    import contextlib
    import concourse.tile as tile
    from concourse import bacc, mybir

    dt16 = getattr(mybir.dt, io_dt_name)
    f32 = mybir.dt.float32
    LRELU = mybir.ActivationFunctionType.Lrelu
    COPY = mybir.ActivationFunctionType.Copy

    nc = bacc.Bacc(trn_type="TRN2", target_bir_lowering=False, debug=False,
                   num_devices=N_CORES)

    xm = nc.declare_dram_parameter("xm", [b_loc, DM], dt16, isOutput=False).ap()
    xtail = nc.declare_dram_parameter("xtail", [TAIL, b_loc], dt16,
                                      isOutput=False).ap()
    w1t = nc.declare_dram_parameter("w1t", [DM, D1], dt16, isOutput=False).ap()
    w1l = nc.declare_dram_parameter("w1l", [TAIL, D1], dt16, isOutput=False).ap()
    w2t = nc.declare_dram_parameter("w2t", [D1, D2], dt16, isOutput=False).ap()
    m3t = nc.declare_dram_parameter("m3t", [D2, D2], dt16, isOutput=False).ap()
    d2t = nc.declare_dram_parameter("d2t", [D2, D1], dt16, isOutput=False).ap()
    d1t = nc.declare_dram_parameter("d1t", [D1, D0], dt16, isOutput=False).ap()
    ident = nc.declare_dram_parameter("ident", [SUB, SUB], dt16, isOutput=False).ap()
    out = nc.declare_dram_parameter("out", [b_loc, D0], dt16, isOutput=True).ap()

    n_tiles = b_loc // TILE
    # row = tile*512 + p*4 + s  (4 rows per partition -> one 0.75MB DMA/tile);
    # xtail is host-permuted so its natural column order matches col = s*128+p.
    x_r = xm.rearrange("(n p s) f -> n p (s f)", p=SUB, s=NSUB)
    out_r = out.rearrange("(n p s) f -> n p (s f)", p=SUB, s=NSUB)
    xt_r = xtail.rearrange("q (n c) -> n q c", c=TILE)

    with tile.TileContext(nc, num_cores=N_CORES, pool_alloc_mode="stack") as tc:
        with (
            tc.tile_pool(name="consts", bufs=1) as consts,
            tc.tile_pool(name="xin", bufs=xin_bufs) as xin,
            tc.tile_pool(name="xtp", bufs=xtp_bufs) as xtp,
            tc.tile_pool(name="acts", bufs=acts_bufs) as acts,
            tc.tile_pool(name="outp", bufs=outp_bufs) as outp,
            tc.tile_pool(name="psT", bufs=2, space="PSUM") as psT,
            tc.tile_pool(name="psMM", bufs=2, space="PSUM") as psMM,
            tc.tile_pool(name="psO", bufs=2, space="PSUM") as psO,
        ):
            # --- constants ---
            w1t_sb = consts.tile([SUB, NKC, D1], dt16)
            nc.sync.dma_start(out=w1t_sb, in_=w1t.rearrange("(c p) m -> p c m", p=SUB))
            w1l_sb = consts.tile([TAIL, D1], dt16)
            nc.sync.dma_start(out=w1l_sb, in_=w1l)
            w2t_sb = consts.tile([D1, D2], dt16)
            nc.sync.dma_start(out=w2t_sb, in_=w2t)
            m3t_sb = consts.tile([D2, D2], dt16)
            nc.sync.dma_start(out=m3t_sb, in_=m3t)
            d2t_sb = consts.tile([D2, D1], dt16)
            nc.sync.dma_start(out=d2t_sb, in_=d2t)
            d1t_sb = consts.tile([D1, D0], dt16)
            nc.sync.dma_start(out=d1t_sb, in_=d1t)
            id_sb = consts.tile([SUB, SUB], dt16)
            nc.sync.dma_start(out=id_sb, in_=ident)

            if compute_only:
                x_const = consts.tile([SUB, NSUB, DM], dt16)
                nc.sync.dma_start(out=x_const, in_=x_r[0])
                xl_const = consts.tile([TAIL, TILE], dt16)
                nc.sync.dma_start(out=xl_const, in_=xt_r[0])

            rep_ctx = (tc.For_i(0, repeat, 1, staggered_reset=staggered)
                       if repeat > 1 else contextlib.nullcontext())
            with rep_ctx:
              for t in range(n_tiles):
                # --- load 512 rows: [128, 4, 768] main + [16, 512] tail ---
                if compute_only:
                    x_sb, xl_sb = x_const, xl_const
                else:
                    x_sb = xin.tile([SUB, NSUB, DM], dt16, tag="x")
                    xl_sb = xin.tile([TAIL, TILE], dt16, tag="xl")
                    if in_dma_eng == "alt":
                        eng = nc.sync if t % 2 == 0 else nc.scalar
                    else:
                        eng = getattr(nc, in_dma_eng)
                    eng.dma_start(out=x_sb, in_=x_r[t])
                    eng.dma_start(out=xl_sb, in_=xt_r[t])

                if dma_only:
                    o_sb = outp.tile([SUB, NSUB, D0], dt16, tag="o")
                    nc.vector.tensor_copy(
                        o_sb.rearrange("p a b -> p (a b)")[:, :NSUB * DM],
                        x_sb.rearrange("p a b -> p (a b)"))
                    getattr(nc, out_dma_eng).dma_start(out=out_r[t], in_=o_sb)
                    continue

                # --- PE-transpose to feature-major: 6 chunks of [128, 512],
                # two chunks per fp16 PSUM bank -> 3 pair copies ---
                xt_sb = []
                for cp in range(NKC // 2):
                    tp = psT.tile([SUB, 2, TILE], dt16, tag="psT")
                    for h in range(2):
                        c = 2 * cp + h
                        for s in range(NSUB):
                            nc.tensor.transpose(
                                out=tp[:, h, s * SUB:(s + 1) * SUB],
                                in_=x_sb[:, s, c * SUB:(c + 1) * SUB],
                                identity=id_sb,
                            )
                    xt = xtp.tile([SUB, 2, TILE], dt16, tag="xt")
                    if xcopy == "scalar":
                        nc.scalar.activation(out=xt, in_=tp, func=COPY)
                    else:
                        nc.vector.tensor_copy(xt, tp)
                    xt_sb.append(xt)

                # --- L1: h1 = lrelu(W1 @ xT)  [128, 512] (6 main + tail) ---
                h1_ps = psMM.tile([D1, TILE], f32, tag="mm")
                for c in range(NKC):
                    nc.tensor.matmul(h1_ps, lhsT=w1t_sb[:, c, :],
                                     rhs=xt_sb[c // 2][:, c % 2, :],
                                     start=(c == 0), stop=False)
                nc.tensor.matmul(h1_ps, lhsT=w1l_sb, rhs=xl_sb,
                                 start=False, stop=True)
                h1_sb = acts.tile([D1, TILE], dt16, tag="h1")
                nc.scalar.activation(out=h1_sb, in_=h1_ps, func=LRELU, alpha=0.01)

                # --- L2: h2 = lrelu(W2 @ h1)  [64, 512] ---
                h2_ps = psMM.tile([D2, TILE], f32, tag="mm")
                nc.tensor.matmul(h2_ps, lhsT=w2t_sb, rhs=h1_sb,
                                 start=True, stop=True)
                h2_sb = acts.tile([D2, TILE], dt16, tag="h2")
                nc.scalar.activation(out=h2_sb, in_=h2_ps, func=LRELU, alpha=0.01)

                # --- L3 folded: g3 = lrelu((d3 @ W3) @ h2)  [64, 512] ---
                g3_ps = psMM.tile([D2, TILE], f32, tag="mm")
                nc.tensor.matmul(g3_ps, lhsT=m3t_sb, rhs=h2_sb,
                                 start=True, stop=True)
                g3_sb = acts.tile([D2, TILE], dt16, tag="g3")
                nc.scalar.activation(out=g3_sb, in_=g3_ps, func=LRELU, alpha=0.01)

                # --- L4: g2 = lrelu(d2 @ g3)  [128, 512] ---
                g2_ps = psMM.tile([D1, TILE], f32, tag="mm")
                nc.tensor.matmul(g2_ps, lhsT=d2t_sb, rhs=g3_sb,
                                 start=True, stop=True)
                g2_sb = acts.tile([D1, TILE], dt16, tag="g2")
                nc.scalar.activation(out=g2_sb, in_=g2_ps, func=LRELU, alpha=0.01)

                # --- L5: out = g2.T @ d1.T, batch-major via stationary swap.
                # Two matmuls into one 2-bank PSUM tile ([:, :392] in bank 0,
                # [:, 512:904] in bank 1), one strided copy out. ---
                o_sb = outp.tile([SUB, NSUB, D0], dt16, tag="o")
                for s in range(NSUB):
                    g2c = g2_sb[:, s * SUB:(s + 1) * SUB]
                    po = psO.tile([SUB, 1024], f32, tag="po")
                    nc.tensor.matmul(po[:, :TILE], lhsT=g2c, rhs=d1t_sb[:, :TILE],
                                     start=True, stop=True)
                    nc.tensor.matmul(po[:, TILE:D0], lhsT=g2c,
                                     rhs=d1t_sb[:, TILE:], start=True, stop=True)
                    o_v = o_sb[:, s, :]
                    if ocopy == "scalar" or (ocopy == "split" and s % 2 == 0):
                        nc.scalar.activation(out=o_v, in_=po[:, :D0], func=COPY)
                    else:
                        nc.vector.tensor_copy(o_v, po[:, :D0])
                getattr(nc, out_dma_eng).dma_start(out=out_r[t], in_=o_sb)

              if compute_only:
                nc.sync.dma_start(out=out_r[0], in_=o_sb)

    nc.finalize()
    return nc


def _host_weights(W1, W2, W3):
    def pinv(W):
        u, s, vh = np.linalg.svd(W.astype(np.float64), full_matrices=False)
        return (vh.T * (1.0 / s)) @ u.T

    d1, d2, d3 = pinv(W1), pinv(W2), pinv(W3)
    f = np.float16
    w1tf = W1.T  # [784, 128]
    return {
        "w1t": np.ascontiguousarray(w1tf[:DM], dtype=f),
        "w1l": np.ascontiguousarray(w1tf[DM:], dtype=f),
        "w2t": np.ascontiguousarray(W2.T, dtype=f),
        "m3t": np.ascontiguousarray((d3 @ W3.astype(np.float64)).T, dtype=f),
        "d2t": np.ascontiguousarray(d2.T, dtype=f),
        "d1t": np.ascontiguousarray(d1.T, dtype=f),
        "ident": np.eye(SUB, dtype=f),
    }


def _in_maps(x, W1, W2, W3):
    x = np.asarray(x, dtype=np.float16)
    w = _host_weights(np.asarray(W1), np.asarray(W2), np.asarray(W3))
    n_tiles = B_LOC // TILE
    maps = []
    for i in range(N_CORES):
        xs = x[i * B_LOC:(i + 1) * B_LOC]
        # tail, feature-major, columns permuted to the transpose order:
        # on-chip column s*128+p of tile t holds batch row t*512 + p*4 + s.
        tl = np.ascontiguousarray(xs[:, DM:].T)          # [16, 8192] natural
        tl = tl.reshape(TAIL, n_tiles, SUB, NSUB)        # [q, t, p, s]
        tl = np.ascontiguousarray(tl.transpose(0, 1, 3, 2))  # [q, t, s, p]
        maps.append({
            "xm": np.ascontiguousarray(xs[:, :DM]),
            "xtail": tl.reshape(TAIL, B_LOC),
            **w,
        })
    return maps


_NC_CACHE = {}


def _get_nc(key=()):
    if key not in _NC_CACHE:
        _NC_CACHE[key] = _build_nc(B_LOC)
    return _NC_CACHE[key]


def kernel(x, W1, W2, W3):
    from concourse.bass_utils import run_bass_kernel_spmd

    in_maps = _in_maps(x, W1, W2, W3)
    nc = _get_nc()
    res = run_bass_kernel_spmd(nc, in_maps, core_ids=list(range(N_CORES)))
    return np.concatenate(
        [res.results[i]["out"] for i in range(N_CORES)], axis=0
    ).astype(np.float32)


# revision 23
# speedup vs baseline: 1.1017x; 1.0022x over previous
"""Trainium2 Bass kernel for the 3-layer MLP encode/decode forward pass.

Computation (B = 65536):
    d_i = pinv(W_i)                       (host, negligible)
    h = lrelu(x @ W1.T)                   [B, 128]
    h = lrelu(h @ W2.T)                   [B, 64]
    h = h @ W3.T                          [B, 16]
    h = lrelu(h @ d3.T)                   [B, 64]   (folded: lrelu((d3@W3) @ h2))
    h = lrelu(h @ d2.T)                   [B, 128]
    out = h @ d1.T                        [B, 784]

Sharding: pure data-parallel — 8 cores x 8192 batch rows each; the tiny
weights (and host-side pinv) are replicated.

All I/O and matmul operands are fp16 (fp32 PSUM accumulation;
end-to-end rel err ~6e-4, well inside the 2e-2 gate), which halves the
DMA traffic vs fp32.  The kernel is jointly bound by HBM traffic
(2 x 12.9 MB/core/pass) and the PE sequencer's LDWEIGHTS+MATMUL issue
stream, so the layer-1 contraction is chunked 6 x 128 + 16: the six
128-column chunks are PE-transposed on chip (24 transpose issues per
512-row tile instead of 28, and 128-wide fp16 stationaries take the
fast-weight-load path), while the 16-column tail is shipped by the
host launcher already feature-major (xtail), column-permuted to match
the batch permutation the on-chip transposes produce.

Per-core layout: activations are kept feature-major ([feat, batch]) so
TensorE contracts over features.  Transposes land two 128-feature
chunks per fp16 PSUM bank so one DVE copy drains 1024 columns.  The
final layer swaps operand roles (stationary = activation tile, moving
= d1.T) so the output lands batch-major in PSUM — no output transpose.

DMA: per 512-row tile, one 0.75MB main load + 16KB tail load in, one
0.8MB store out (4 batch rows per partition — batch order inside a
tile is permuted; the permutation cancels between the input transposes
/ permuted tail and the output writeback).
"""

import numpy as np

B = 65536
N_CORES = 8
B_LOC = B // N_CORES  # 8192
D0, D1, D2, D3 = 784, 128, 64, 16
DM = 768           # 6 x 128 main feature chunks
NKC = DM // 128    # 6
TAIL = D0 - DM     # 16
TILE = 512         # moving free dim per matmul (one fp32 PSUM bank)
SUB = 128          # batch sub-tile (partition dim of x / out tiles)
NSUB = TILE // SUB  # 4
HALF = D0 // 2     # 392

IO_DT = "float16"  # on-chip + DRAM dtype for x / weights / out


def _build_nc(b_loc=B_LOC, repeat=1, io_dt_name=IO_DT,
              in_dma_eng="sync", out_dma_eng="sync",
              xin_bufs=4, outp_bufs=4, xtp_bufs=8, acts_bufs=2,
              ocopy="split", xcopy="vector", staggered=True,
              dma_only=False, compute_only=False):
    import contextlib
    import concourse.tile as tile
    from concourse import bacc, mybir

    dt16 = getattr(mybir.dt, io_dt_name)
    f32 = mybir.dt.float32
    LRELU = mybir.ActivationFunctionType.Lrelu
    COPY = mybir.ActivationFunctionType.Copy

    nc = bacc.Bacc(trn_type="TRN2", target_bir_lowering=False, debug=False,
                   num_devices=N_CORES)

    xm = nc.declare_dram_parameter("xm", [b_loc, DM], dt16, isOutput=False).ap()
    xtail = nc.declare_dram_parameter("xtail", [TAIL, b_loc], dt16,
                                      isOutput=False).ap()
    w1t = nc.declare_dram_parameter("w1t", [DM, D1], dt16, isOutput=False).ap()
    w1l = nc.declare_dram_parameter("w1l", [TAIL, D1], dt16, isOutput=False).ap()
    w2t = nc.declare_dram_parameter("w2t", [D1, D2], dt16, isOutput=False).ap()
    m3t = nc.declare_dram_parameter("m3t", [D2, D2], dt16, isOutput=False).ap()
    d2t = nc.declare_dram_parameter("d2t", [D2, D1], dt16, isOutput=False).ap()
    d1t = nc.declare_dram_parameter("d1t", [D1, D0], dt16, isOutput=False).ap()
    ident = nc.declare_dram_parameter("ident", [SUB, SUB], dt16, isOutput=False).ap()
    out = nc.declare_dram_parameter("out", [b_loc, D0], dt16, isOutput=True).ap()

    n_tiles = b_loc // TILE
    # row = tile*512 + p*4 + s  (4 rows per partition -> one 0.75MB DMA/tile);
    # xtail is host-permuted so its natural column order matches col = s*128+p.
    x_r = xm.rearrange("(n p s) f -> n p (s f)", p=SUB, s=NSUB)
    out_r = out.rearrange("(n p s) f -> n p (s f)", p=SUB, s=NSUB)
    xt_r = xtail.rearrange("q (n c) -> n q c", c=TILE)

    with tile.TileContext(nc, num_cores=N_CORES, pool_alloc_mode="stack") as tc:
        with (
            tc.tile_pool(name="consts", bufs=1) as consts,
            tc.tile_pool(name="xin", bufs=xin_bufs) as xin,
            tc.tile_pool(name="xtp", bufs=xtp_bufs) as xtp,
            tc.tile_pool(name="acts", bufs=acts_bufs) as acts,
            tc.tile_pool(name="outp", bufs=outp_bufs) as outp,
            tc.tile_pool(name="psT", bufs=2, space="PSUM") as psT,
            tc.tile_pool(name="psMM", bufs=2, space="PSUM") as psMM,
            tc.tile_pool(name="psO", bufs=2, space="PSUM") as psO,
        ):
            # --- constants ---
            w1t_sb = consts.tile([SUB, NKC, D1], dt16)
            nc.sync.dma_start(out=w1t_sb, in_=w1t.rearrange("(c p) m -> p c m", p=SUB))
            w1l_sb = consts.tile([TAIL, D1], dt16)
            nc.sync.dma_start(out=w1l_sb, in_=w1l)
            w2t_sb = consts.tile([D1, D2], dt16)
            nc.sync.dma_start(out=w2t_sb, in_=w2t)
            m3t_sb = consts.tile([D2, D2], dt16)
            nc.sync.dma_start(out=m3t_sb, in_=m3t)
            d2t_sb = consts.tile([D2, D1], dt16)
            nc.sync.dma_start(out=d2t_sb, in_=d2t)
            d1t_sb = consts.tile([D1, D0], dt16)
            nc.sync.dma_start(out=d1t_sb, in_=d1t)
            id_sb = consts.tile([SUB, SUB], dt16)
            nc.sync.dma_start(out=id_sb, in_=ident)

            if compute_only:
                x_const = consts.tile([SUB, NSUB, DM], dt16)
                nc.sync.dma_start(out=x_const, in_=x_r[0])
                xl_const = consts.tile([TAIL, TILE], dt16)
                nc.sync.dma_start(out=xl_const, in_=xt_r[0])

            rep_ctx = (tc.For_i(0, repeat, 1, staggered_reset=staggered)
                       if repeat > 1 else contextlib.nullcontext())
            with rep_ctx:
              for t in range(n_tiles):
                # --- load 512 rows: [128, 4, 768] main + [16, 512] tail ---
                if compute_only:
                    x_sb, xl_sb = x_const, xl_const
                else:
                    x_sb = xin.tile([SUB, NSUB, DM], dt16, tag="x")
                    xl_sb = xin.tile([TAIL, TILE], dt16, tag="xl")
                    if in_dma_eng == "alt":
                        eng = nc.sync if t % 2 == 0 else nc.scalar
                    else:
                        eng = getattr(nc, in_dma_eng)
                    eng.dma_start(out=x_sb, in_=x_r[t])
                    eng.dma_start(out=xl_sb, in_=xt_r[t])

                if dma_only:
                    o_sb = outp.tile([SUB, NSUB, D0], dt16, tag="o")
                    nc.vector.tensor_copy(
                        o_sb.rearrange("p a b -> p (a b)")[:, :NSUB * DM],
                        x_sb.rearrange("p a b -> p (a b)"))
                    getattr(nc, out_dma_eng).dma_start(out=out_r[t], in_=o_sb)
                    continue

                # --- PE-transpose to feature-major: 6 chunks of [128, 512],
                # two chunks per fp16 PSUM bank -> 3 pair copies ---
                xt_sb = []
                for cp in range(NKC // 2):
                    tp = psT.tile([SUB, 2, TILE], dt16, tag="psT")
                    for h in range(2):
                        c = 2 * cp + h
                        for s in range(NSUB):
                            nc.tensor.transpose(
                                out=tp[:, h, s * SUB:(s + 1) * SUB],
                                in_=x_sb[:, s, c * SUB:(c + 1) * SUB],
                                identity=id_sb,
                            )
                    xt = xtp.tile([SUB, 2, TILE], dt16, tag="xt")
                    if xcopy == "scalar":
                        nc.scalar.activation(out=xt, in_=tp, func=COPY)
                    else:
                        nc.vector.tensor_copy(xt, tp)
                    xt_sb.append(xt)

                # --- L1: h1 = lrelu(W1 @ xT)  [128, 512] (6 main + tail) ---
                h1_ps = psMM.tile([D1, TILE], f32, tag="mm")
                for c in range(NKC):
                    nc.tensor.matmul(h1_ps, lhsT=w1t_sb[:, c, :],
                                     rhs=xt_sb[c // 2][:, c % 2, :],
                                     start=(c == 0), stop=False)
                nc.tensor.matmul(h1_ps, lhsT=w1l_sb, rhs=xl_sb,
                                 start=False, stop=True)
                h1_sb = acts.tile([D1, TILE], dt16, tag="h1")
                nc.scalar.activation(out=h1_sb, in_=h1_ps, func=LRELU, alpha=0.01)

                # --- L2: h2 = lrelu(W2 @ h1)  [64, 512] ---
                h2_ps = psMM.tile([D2, TILE], f32, tag="mm")
                nc.tensor.matmul(h2_ps, lhsT=w2t_sb, rhs=h1_sb,
                                 start=True, stop=True)
                h2_sb = acts.tile([D2, TILE], dt16, tag="h2")
                nc.scalar.activation(out=h2_sb, in_=h2_ps, func=LRELU, alpha=0.01)

                # --- L3 folded: g3 = lrelu((d3 @ W3) @ h2)  [64, 512] ---
                g3_ps = psMM.tile([D2, TILE], f32, tag="mm")
                nc.tensor.matmul(g3_ps, lhsT=m3t_sb, rhs=h2_sb,
                                 start=True, stop=True)
                g3_sb = acts.tile([D2, TILE], dt16, tag="g3")
                nc.scalar.activation(out=g3_sb, in_=g3_ps, func=LRELU, alpha=0.01)

                # --- L4: g2 = lrelu(d2 @ g3)  [128, 512] ---
                g2_ps = psMM.tile([D1, TILE], f32, tag="mm")
                nc.tensor.matmul(g2_ps, lhsT=d2t_sb, rhs=g3_sb,
                                 start=True, stop=True)
                g2_sb = acts.tile([D1, TILE], dt16, tag="g2")
                nc.scalar.activation(out=g2_sb, in_=g2_ps, func=LRELU, alpha=0.01)

                # --- L5: out = g2.T @ d1.T, batch-major via stationary swap.
                # Two matmuls into one 2-bank PSUM tile ([:, :392] in bank 0,
                # [:, 512:904] in bank 1), one strided copy out. ---
                o_sb = outp.tile([SUB, NSUB, D0], dt16, tag="o")
                for s in range(NSUB):
                    g2c = g2_sb[:, s * SUB:(s + 1) * SUB]
                    po = psO.tile([SUB, 1024], f32, tag="po")
                    nc.tensor.matmul(po[:, :TILE], lhsT=g2c, rhs=d1t_sb[:, :TILE],
                                     start=True, stop=True)
                    nc.tensor.matmul(po[:, TILE:D0], lhsT=g2c,
                                     rhs=d1t_sb[:, TILE:], start=True, stop=True)
                    o_v = o_sb[:, s, :]
                    if ocopy == "scalar" or (ocopy == "split" and s % 2 == 0):
                        nc.scalar.activation(out=o_v, in_=po[:, :D0], func=COPY)
                    else:
                        nc.vector.tensor_copy(o_v, po[:, :D0])
                getattr(nc, out_dma_eng).dma_start(out=out_r[t], in_=o_sb)

              if compute_only:
                nc.sync.dma_start(out=out_r[0], in_=o_sb)

    nc.finalize()
    return nc


def _host_weights(W1, W2, W3):
    def pinv(W):
        u, s, vh = np.linalg.svd(W.astype(np.float64), full_matrices=False)
        return (vh.T * (1.0 / s)) @ u.T

    d1, d2, d3 = pinv(W1), pinv(W2), pinv(W3)
    f = np.float16
    w1tf = W1.T  # [784, 128]
    return {
        "w1t": np.ascontiguousarray(w1tf[:DM], dtype=f),
        "w1l": np.ascontiguousarray(w1tf[DM:], dtype=f),
        "w2t": np.ascontiguousarray(W2.T, dtype=f),
        "m3t": np.ascontiguousarray((d3 @ W3.astype(np.float64)).T, dtype=f),
        "d2t": np.ascontiguousarray(d2.T, dtype=f),
        "d1t": np.ascontiguousarray(d1.T, dtype=f),
        "ident": np.eye(SUB, dtype=f),
    }


def _in_maps(x, W1, W2, W3):
    x = np.asarray(x, dtype=np.float16)
    w = _host_weights(np.asarray(W1), np.asarray(W2), np.asarray(W3))
    n_tiles = B_LOC // TILE
    maps = []
    for i in range(N_CORES):
        xs = x[i * B_LOC:(i + 1) * B_LOC]
        # tail, feature-major, columns permuted to the transpose order:
        # on-chip column s*128+p of tile t holds batch row t*512 + p*4 + s.
        tl = np.ascontiguousarray(xs[:, DM:].T)          # [16, 8192] natural
        tl = tl.reshape(TAIL, n_tiles, SUB, NSUB)        # [q, t, p, s]
        tl = np.ascontiguousarray(tl.transpose(0, 1, 3, 2))  # [q, t, s, p]
        maps.append({
            "xm": np.ascontiguousarray(xs[:, :DM]),
            "xtail": tl.reshape(TAIL, B_LOC),
            **w,
        })
    return maps


_NC_CACHE = {}


def _get_nc(key=()):
    if key not in _NC_CACHE:
        _NC_CACHE[key] = _build_nc(B_LOC)
    return _NC_CACHE[key]


def kernel(x, W1, W2, W3):
    from concourse.bass_utils import run_bass_kernel_spmd

    in_maps = _in_maps(x, W1, W2, W3)
    nc = _get_nc()
    res = run_bass_kernel_spmd(nc, in_maps, core_ids=list(range(N_CORES)))
    return np.concatenate(
        [res.results[i]["out"] for i in range(N_CORES)], axis=0
    ).astype(np.float32)
